# revision 1
# baseline (speedup 1.0000x reference)
"""Weighted-BCE (Hanning) loss on 8 Trainium2 NeuronCores.

Math: reference loss per image i with box top-left (y0,x0) (the 33x33 block of
1.0s in target; (0,0) when absent) and hann window h (S = sum(h), nnz = count
of h != 0, n_zero = H*W - nnz):

    weights = h/(2S) on box positions where h != 0, else 1/(2*n_zero)
    bce     = softplus(pred) - pred*target
    loss_i  = sum_box(bce*h)/(2S) + (T_i - Z_i)/(2*n_zero)
      T_i   = sum_all(softplus(pred)) - sum_all(pred*target)
      Z_i   = sum_box(bce * (h != 0))

Device computes the O(B*H*W) part: per-image softplus total (ACT Exp + Ln with
fused accumulate) and per-row maxima of target (to locate the box rows).
Host computes the O(B*33^2) box tail and the final scalar combine.

Sharding: pure data parallel, 6 images per core. Each image [512,512] is
viewed as [64,4096] (partition = 8-row group); images are processed in pairs
as [128,4096] tiles (image 2p in partitions 0-63, image 2p+1 in 64-127).
"""

import numpy as np

B, H, W, KW = 48, 512, 512, 33
N_CORES = 8
IMGS_PER_CORE = B // N_CORES  # 6
PAIRS = IMGS_PER_CORE // 2  # 3
ROWS_PER_PART = 8  # rows of one image per partition in the [128,4096] view
OUT_COLS = PAIRS + PAIRS * ROWS_PER_PART  # 3 softplus-sum cols + 24 rowmax cols

_CACHE = {}


def _build_bass(n_iters: int = 1):
    """Build+compile the per-core bass program. n_iters>1 repeats the body
    (same inputs) for wall-clock device timing; outputs are identical."""
    import concourse.bass as bass
    import concourse.tile as tile
    from concourse import bacc, mybir

    f32 = mybir.dt.float32
    bf16 = mybir.dt.bfloat16
    nc = bacc.Bacc("TRN2", target_bir_lowering=False, debug=False, num_devices=N_CORES)
    pred_ap = nc.dram_tensor(
        "pred", [PAIRS * 128, 4096], bf16, kind="ExternalInput"
    ).ap()
    tgt_ap = nc.dram_tensor(
        "target", [PAIRS * 128, 4096], bf16, kind="ExternalInput"
    ).ap()
    out_ap = nc.dram_tensor("out", [128, OUT_COLS], f32, kind="ExternalOutput").ap()

    with tile.TileContext(nc) as tc:
        with (
            tc.tile_pool(name="pin", bufs=3) as pin,
            tc.tile_pool(name="tin", bufs=3) as tin,
            tc.tile_pool(name="mid", bufs=2) as mid,
            tc.tile_pool(name="obuf", bufs=1) as obuf,
        ):
            ob = obuf.tile([128, OUT_COLS], f32)

            def body(_iv):
                for p in range(PAIRS):
                    tx = pin.tile([128, 4096], bf16, tag="pred")
                    nc.sync.dma_start(tx[:], pred_ap[bass.ts(p, 128), :])
                    tt = tin.tile([128, 4096], bf16, tag="tgt")
                    nc.sync.dma_start(tt[:], tgt_ap[bass.ts(p, 128), :])
                    te = mid.tile([128, 4096], f32, tag="exp")
                    nc.scalar.activation(te[:], tx[:], mybir.ActivationFunctionType.Exp)
                    ts = mid.tile([128, 4096], f32, tag="sp")
                    nc.scalar.activation(
                        ts[:],
                        te[:],
                        mybir.ActivationFunctionType.Ln,
                        bias=1.0,
                        accum_out=ob[:, p : p + 1],
                    )
                    rm_lo = PAIRS + p * ROWS_PER_PART
                    nc.vector.tensor_reduce(
                        ob[:, rm_lo : rm_lo + ROWS_PER_PART],
                        tt[:].rearrange("q (r w) -> q r w", r=ROWS_PER_PART),
                        axis=mybir.AxisListType.X,
                        op=mybir.AluOpType.max,
                    )

            if n_iters == 1:
                body(0)
            else:
                tc.For_i_unrolled(0, n_iters, 1, body, max_unroll=8)
            nc.sync.dma_start(out_ap[:], ob[:])
    nc.compile()
    return nc


def _get_nc(n_iters: int = 1):
    if n_iters not in _CACHE:
        _CACHE[n_iters] = _build_bass(n_iters)
    return _CACHE[n_iters]


def _shard_inputs(pred, target):
    """bf16 per-core shards in the [384, 4096] device layout.

    bf16 is exact for the 0/1 target mask; for pred it perturbs each softplus
    term by ~1e-3 relative, which averages out to ~3e-6 relative on the
    262144-element per-image sum (verified against the f32 reference).
    """
    import ml_dtypes

    predb = np.ascontiguousarray(pred).astype(ml_dtypes.bfloat16)
    tgtb = np.ascontiguousarray(target).astype(ml_dtypes.bfloat16)
    in_maps = [
        {
            "pred": predb[c * IMGS_PER_CORE : (c + 1) * IMGS_PER_CORE].reshape(
                PAIRS * 128, 4096
            ),
            "target": tgtb[c * IMGS_PER_CORE : (c + 1) * IMGS_PER_CORE].reshape(
                PAIRS * 128, 4096
            ),
        }
        for c in range(N_CORES)
    ]
    tgt_lossless = np.array_equal(tgtb.astype(np.float32), target)
    return in_maps, tgt_lossless


def _device_sums(pred, target):
    """Run the 8-core SPMD kernel. Returns (sp_total[B], rowmax[B,512] or None)."""
    from concourse.bass_utils import run_bass_kernel_spmd

    nc = _get_nc(1)
    in_maps, tgt_lossless = _shard_inputs(pred, target)
    res = run_bass_kernel_spmd(nc, in_maps, list(range(N_CORES))).results

    sp_total = np.empty(B, dtype=np.float64)
    rowmax = np.empty((B, H), dtype=np.float32)
    for c in range(N_CORES):
        out = res[c]["out"]  # [128, OUT_COLS]
        for p in range(PAIRS):
            sp_col = out[:, p]
            rm = out[:, PAIRS + p * ROWS_PER_PART : PAIRS + (p + 1) * ROWS_PER_PART]
            for half in range(2):
                img = c * IMGS_PER_CORE + p * 2 + half
                sp_total[img] = sp_col[half * 64 : (half + 1) * 64].sum(
                    dtype=np.float64
                )
                # partition q, col r -> image row 8*(q%64) + r
                rowmax[img] = rm[half * 64 : (half + 1) * 64].reshape(H)
    return sp_total, (rowmax if tgt_lossless else None)


def kernel(pred, target, hann_kernel):
    pred = np.asarray(pred, dtype=np.float32)
    target = np.asarray(target, dtype=np.float32)
    hann = np.asarray(hann_kernel, dtype=np.float32)

    sp_total, rowmax = _device_sums(pred, target)

    hann64 = hann.astype(np.float64)
    nzmask = hann64 != 0.0
    S = hann64.sum()
    n_zero = H * W - int(nzmask.sum())

    losses = np.empty(B, dtype=np.float64)
    for i in range(B):
        if rowmax is not None:
            has1 = rowmax[i] == 1.0
        else:  # rare fallback: target not bf16-lossless, scan f32 rows on host
            has1 = (target[i] == 1.0).any(axis=1)
        y0 = int(np.argmax(has1))
        x0 = int(np.argmax(target[i, y0] == 1.0))
        # dynamic_update_slice clamps the window to stay in-bounds
        y0 = min(y0, H - KW)
        x0 = min(x0, W - KW)
        pp = pred[i, y0 : y0 + KW, x0 : x0 + KW].astype(np.float64)
        tt = target[i, y0 : y0 + KW, x0 : x0 + KW].astype(np.float64)
        pt_box = pp * tt
        bce_box = np.logaddexp(0.0, pp) - pt_box
        A = (bce_box * hann64).sum()
        Z = bce_box[nzmask].sum()
        T_i = sp_total[i] - pt_box.sum()
        losses[i] = A / (2.0 * S) + (T_i - Z) / (2.0 * n_zero)

    return np.array(losses.mean(), dtype=np.float32)



# revision 7
# speedup vs baseline: 1.6697x; 1.6697x over previous
"""Weighted-BCE (Hanning) loss on 8 Trainium2 NeuronCores.

Math: reference loss per image i with box top-left (y0,x0) (the 33x33 block of
1.0s in target; (0,0) when absent) and hann window h (S = sum(h), nnz = count
of h != 0, n_zero = H*W - nnz):

    weights = h/(2S) on box positions where h != 0, else 1/(2*n_zero)
    bce     = softplus(pred) - pred*target
    loss_i  = sum_box(bce*h)/(2S) + (T_i - Z_i)/(2*n_zero)
      T_i   = sum_all(softplus(pred)) - sum_box(pred)
      Z_i   = sum_box(bce * (h != 0))

Only mean_i(loss_i) is required, and mean_i T_i depends only on the GLOBAL
softplus sum, so the device computes exactly one O(B*H*W) quantity: the sum of
softplus(pred) over every pixel. The box tail (A_i, Z_i, sum_box(pred)) is
O(B*33^2) and computed on host, with the box located by a single argmax over
target (first 1.0 in row-major order is the box's top-left corner; (0,0) when
target is all zeros, matching the reference's argmax-of-zeros behavior).

Device kernel per core (pure data parallel, 6 images per core, 1,572,864
elements as fp8 e4m3 viewed as [128, 12288]):

There is no hardware Softplus activation table, so two full ACT table passes
(Exp then Ln) would cost ~2N scalar-engine cycles. Instead use
    sum_j ln(1+e^{x_j}) = ln(prod_j (1+e^{x_j}))
over groups of G=16 elements: ACT computes v = exp(x - ln2) = e^x/2 (one full
N-cycle pass; the /2 keeps group products inside bf16/f32 exponent range),
DVE adds 0.5 (w = (1+e^x)/2, 4x mode) and multiplies pairs in a 4-level tree
(2x mode), and ACT's second table pass is Ln over only N/16 elements with a
fused accum_out per-partition sum. Host adds back the N*ln2 rescale. ACT cost
is ~N + N/16 cycles; DVE (~0.72N cycles at 0.96GHz) and the fp8 DMA hide
underneath. Exp and Ln share one activation table set
(natural_log_exp_and_others), so no per-iteration table switches.
"""

import numpy as np

B, H, W, KW = 48, 512, 512, 33
N_CORES = 8
IMGS_PER_CORE = B // N_CORES  # 6
FLAT = IMGS_PER_CORE * H * W  # 1,572,864 elements per core
P = 128
FD = FLAT // P  # 12288 elements per partition
G = 16  # product-group size
LN2 = 0.6931471805599453

_CACHE = {}


def _build_bass(n_iters: int = 1):
    """Build+compile the per-core bass program. n_iters>1 repeats the body
    (same inputs) for wall-clock device timing; outputs are identical."""
    import concourse.bass as bass
    import concourse.tile as tile
    from concourse import bacc, mybir

    f32 = mybir.dt.float32
    bf16 = mybir.dt.bfloat16
    f16 = mybir.dt.float16
    f8 = mybir.dt.float8e4
    mult = mybir.AluOpType.mult
    bypass = mybir.AluOpType.bypass
    add = mybir.AluOpType.add
    nc = bacc.Bacc("TRN2", target_bir_lowering=False, debug=False, num_devices=N_CORES)
    # Register -ln2 as a const AP so it can be used as an activation bias.
    _c = nc.alloc_sbuf_tensor("const-f32-negln2", [128, 1], f32)
    nc.gpsimd.memset(_c.ap(), -LN2)
    nc.const_aps.aps[(f32, -LN2)] = _c.ap()
    nc.all_engine_barrier()
    pred_ap = nc.dram_tensor("pred", [P, FD], f8, kind="ExternalInput").ap()
    out_ap = nc.dram_tensor("out", [P, 1], f32, kind="ExternalOutput").ap()

    with tile.TileContext(nc) as tc:
        with (
            tc.tile_pool(name="pin", bufs=2) as pin,
            tc.tile_pool(name="vbuf", bufs=2) as vbuf,
            tc.tile_pool(name="tree", bufs=2) as tree,
            tc.tile_pool(name="obuf", bufs=1) as obuf,
        ):
            ob = obuf.tile([P, 1], f32)

            def body(_iv):
                tx = pin.tile([P, FD], f8, tag="pred")
                nc.sync.dma_start(tx[:], pred_ap[:])
                v = vbuf.tile([P, FD], f16, tag="v")
                # v = exp(x - ln2) = e^x / 2  (f16: finer mantissa than bf16
                # keeps the downstream +0.5 / product rounding bias small; the
                # pre-tree value range [2e-3, 123] fits f16 comfortably)
                nc.scalar.activation(
                    v[:], tx[:], mybir.ActivationFunctionType.Exp, bias=-LN2
                )
                h = FD // 2
                # wb = v_hi + 0.5 (4x DVE mode); the lo half's +0.5 is fused
                # into L1's fp32 ALU below, where it is exact.
                wb = tree.tile([P, h], f16, tag="wb")
                nc.vector.tensor_scalar_add(wb[:], v[:, h:], 0.5)
                # L1: (v_lo + 0.5) * wb  -> f16 (pair products <= ~1.5e4)
                t1 = tree.tile([P, h], f16, tag="t1")
                nc.vector.scalar_tensor_tensor(
                    t1[:], v[:, :h], 0.5, wb[:], add, mult
                )
                # L2, L3 in bf16 (needs exponent range), L4 -> f32 for the Ln
                h //= 2
                t2 = tree.tile([P, h], bf16, tag="t2")
                nc.vector.scalar_tensor_tensor(
                    t2[:], t1[:, :h], 0.0, t1[:, h:], bypass, mult
                )
                h //= 2
                t3 = tree.tile([P, h], bf16, tag="t3")
                nc.vector.scalar_tensor_tensor(
                    t3[:], t2[:, :h], 0.0, t2[:, h:], bypass, mult
                )
                h //= 2
                t4 = tree.tile([P, h], f32, tag="t4")
                nc.vector.scalar_tensor_tensor(
                    t4[:], t3[:, :h], 0.0, t3[:, h:], bypass, mult
                )
                # ln of group-of-16 products, accumulated per partition
                lnout = tree.tile([P, h], bf16, tag="lnout")
                nc.scalar.activation(
                    lnout[:],
                    t4[:],
                    mybir.ActivationFunctionType.Ln,
                    accum_out=ob[:],
                )

            if n_iters == 1:
                body(0)
            else:
                tc.For_i_unrolled(0, n_iters, 1, body, max_unroll=8)
            nc.sync.dma_start(out_ap[:], ob[:])
    nc.compile()
    return nc


def _get_nc(n_iters: int = 1):
    if n_iters not in _CACHE:
        _CACHE[n_iters] = _build_bass(n_iters)
    return _CACHE[n_iters]


def _shard_inputs(pred):
    """fp8 e4m3 per-core shards in the [128, 12288] device layout."""
    import ml_dtypes

    pred8 = np.ascontiguousarray(pred).astype(ml_dtypes.float8_e4m3)
    return [
        {"pred": pred8[c * IMGS_PER_CORE : (c + 1) * IMGS_PER_CORE].reshape(P, FD)}
        for c in range(N_CORES)
    ]


def _device_softplus_sum(pred):
    """Run the 8-core SPMD kernel; return the global sum of softplus(pred)."""
    from concourse.bass_utils import run_bass_kernel_spmd

    nc = _get_nc(1)
    in_maps = _shard_inputs(pred)
    res = run_bass_kernel_spmd(nc, in_maps, list(range(N_CORES))).results
    # device accumulated ln(prod (1+e^x)/2) = sum softplus(x) - count*ln2
    dev = sum(res[c]["out"].astype(np.float64).sum() for c in range(N_CORES))
    return float(dev) + B * H * W * LN2


def kernel(pred, target, hann_kernel):
    pred = np.asarray(pred, dtype=np.float32)
    target = np.asarray(target, dtype=np.float32)
    hann = np.asarray(hann_kernel, dtype=np.float32)

    sp_global = _device_softplus_sum(pred)

    hann64 = hann.astype(np.float64)
    nzmask = hann64 != 0.0
    S = hann64.sum()
    n_zero = H * W - int(nzmask.sum())

    # Box top-left: first 1.0 of each image in row-major order (0,0 if none).
    flat_idx = np.argmax(target.reshape(B, -1) == 1.0, axis=1)
    tail = 0.0
    for i in range(B):
        y0, x0 = divmod(int(flat_idx[i]), W)
        # dynamic_update_slice clamps the window to stay in-bounds
        y0 = min(y0, H - KW)
        x0 = min(x0, W - KW)
        pp = pred[i, y0 : y0 + KW, x0 : x0 + KW].astype(np.float64)
        tt = target[i, y0 : y0 + KW, x0 : x0 + KW].astype(np.float64)
        pt_box = pp * tt
        bce_box = np.logaddexp(0.0, pp) - pt_box
        A = (bce_box * hann64).sum()
        Z = bce_box[nzmask].sum()
        tail += A / (2.0 * S) - (Z + pt_box.sum()) / (2.0 * n_zero)

    loss = tail / B + (sp_global / B) / (2.0 * n_zero)
    return np.array(loss, dtype=np.float32)


# revision 9
# speedup vs baseline: 1.7208x; 1.0306x over previous
"""Weighted-BCE (Hanning) loss on 8 Trainium2 NeuronCores.

Math: reference loss per image i with box top-left (y0,x0) (the 33x33 block of
1.0s in target; (0,0) when absent) and hann window h (S = sum(h), nnz = count
of h != 0, n_zero = H*W - nnz):

    weights = h/(2S) on box positions where h != 0, else 1/(2*n_zero)
    bce     = softplus(pred) - pred*target
    loss_i  = sum_box(bce*h)/(2S) + (T_i - Z_i)/(2*n_zero)
      T_i   = sum_all(softplus(pred)) - sum_box(pred)
      Z_i   = sum_box(bce * (h != 0))

Only mean_i(loss_i) is required, and mean_i T_i depends only on the GLOBAL
softplus sum, so the device computes exactly one O(B*H*W) quantity: the sum of
softplus(pred) over every pixel. The box tail (A_i, Z_i, sum_box(pred)) is
O(B*33^2) and computed on host, with the box located by a single argmax over
target (first 1.0 in row-major order is the box's top-left corner; (0,0) when
target is all zeros, matching the reference's argmax-of-zeros behavior).

Device kernel per core (pure data parallel, 6 images per core, 1,572,864
elements as fp8 e4m3 viewed as [128, 12288]):

There is no hardware Softplus activation table, so two full ACT table passes
(Exp then Ln) would cost ~2N scalar-engine cycles. Instead use
    sum_j ln(1+e^{x_j}) = ln(prod_j (1+e^{x_j}))
over groups of G=16 elements: ACT computes v = exp(x - ln2) = e^x/2 (one full
N-cycle pass; the /2 keeps group products inside bf16/f32 exponent range),
DVE adds 0.5 (w = (1+e^x)/2, 4x mode) and multiplies pairs in a 4-level tree
(2x mode), and ACT's second table pass is Ln over only N/16 elements with a
fused accum_out per-partition sum. Host adds back the N*ln2 rescale. ACT cost
is ~N + N/16 cycles; DVE (~0.72N cycles at 0.96GHz) and the fp8 DMA hide
underneath. Exp and Ln share one activation table set
(natural_log_exp_and_others), so no per-iteration table switches.
"""

import numpy as np

B, H, W, KW = 48, 512, 512, 33
N_CORES = 8
IMGS_PER_CORE = B // N_CORES  # 6
FLAT = IMGS_PER_CORE * H * W  # 1,572,864 elements per core
P = 128
FD = FLAT // P  # 12288 elements per partition
G = 16  # product-group size
LN2 = 0.6931471805599453

_CACHE = {}


def _build_bass(n_iters: int = 1):
    """Build+compile the per-core bass program. n_iters>1 repeats the body
    (same inputs) for wall-clock device timing; outputs are identical."""
    import concourse.bass as bass
    import concourse.tile as tile
    from concourse import bacc, mybir

    f32 = mybir.dt.float32
    bf16 = mybir.dt.bfloat16
    f16 = mybir.dt.float16
    f8 = mybir.dt.float8e4
    mult = mybir.AluOpType.mult
    bypass = mybir.AluOpType.bypass
    add = mybir.AluOpType.add
    nc = bacc.Bacc("TRN2", target_bir_lowering=False, debug=False, num_devices=N_CORES)
    # Register -ln2 as a const AP so it can be used as an activation bias.
    _c = nc.alloc_sbuf_tensor("const-f32-negln2", [128, 1], f32)
    nc.gpsimd.memset(_c.ap(), -LN2)
    nc.const_aps.aps[(f32, -LN2)] = _c.ap()
    nc.all_engine_barrier()
    pred_ap = nc.dram_tensor("pred", [P, FD], f8, kind="ExternalInput").ap()
    out_ap = nc.dram_tensor("out", [P, 1], f32, kind="ExternalOutput").ap()

    with tile.TileContext(nc) as tc:
        with (
            tc.tile_pool(name="pin", bufs=2) as pin,
            tc.tile_pool(name="vbuf", bufs=2) as vbuf,
            tc.tile_pool(name="tree", bufs=2) as tree,
            tc.tile_pool(name="t4p", bufs=3) as t4p,
            tc.tile_pool(name="obuf", bufs=1) as obuf,
        ):
            ob = obuf.tile([P, 1], f32)
            # Software pipelining: iteration i's Ln consumes iteration i-1's
            # tree output, so the scalar engine's stream is Exp_i, Ln_{i-1},
            # Exp_{i+1}, ... and never stalls on the DVE chain. Prime with a
            # tile of ones (ln(1) accumulates 0; overwritten by later Lns).
            t4_init = t4p.tile([P, FD // G], bf16, tag="t4")
            nc.vector.memset(t4_init[:], 1.0)
            prev_t4 = [t4_init]

            def body(_iv):
                tx = pin.tile([P, FD], f8, tag="pred")
                nc.sync.dma_start(tx[:], pred_ap[:])
                v = vbuf.tile([P, FD], f16, tag="v")
                # v = exp(x - ln2) = e^x / 2  (f16: finer mantissa than bf16
                # keeps the downstream +0.5 / product rounding bias small; the
                # pre-tree value range [2e-3, 123] fits f16 comfortably)
                nc.scalar.activation(
                    v[:], tx[:], mybir.ActivationFunctionType.Exp, bias=-LN2
                )
                # ln of the PREVIOUS iteration's group products (lagged; its
                # DVE chain finished during this iteration's Exp).
                pt = prev_t4[0]
                lnout = tree.tile([P, FD // G], bf16, tag="lnout")
                nc.scalar.activation(
                    lnout[:],
                    pt[:],
                    mybir.ActivationFunctionType.Ln,
                    accum_out=ob[:],
                )
                h = FD // 2
                # wb = v_hi + 0.5 (4x DVE mode); the lo half's +0.5 is fused
                # into L1's fp32 ALU below, where it is exact.
                wb = tree.tile([P, h], f16, tag="wb")
                nc.vector.tensor_scalar_add(wb[:], v[:, h:], 0.5)
                # L1: (v_lo + 0.5) * wb  -> f16 (pair products <= ~1.5e4)
                t1 = tree.tile([P, h], f16, tag="t1")
                nc.vector.scalar_tensor_tensor(
                    t1[:], v[:, :h], 0.5, wb[:], add, mult
                )
                # L2..L4 in bf16 (range needs the 8-bit exponent)
                h //= 2
                t2 = tree.tile([P, h], bf16, tag="t2")
                nc.vector.scalar_tensor_tensor(
                    t2[:], t1[:, :h], 0.0, t1[:, h:], bypass, mult
                )
                h //= 2
                t3 = tree.tile([P, h], bf16, tag="t3")
                nc.vector.scalar_tensor_tensor(
                    t3[:], t2[:, :h], 0.0, t2[:, h:], bypass, mult
                )
                h //= 2
                t4 = t4p.tile([P, h], bf16, tag="t4")
                nc.vector.scalar_tensor_tensor(
                    t4[:], t3[:, :h], 0.0, t3[:, h:], bypass, mult
                )
                prev_t4[0] = t4

            if n_iters == 1:
                body(0)
            else:
                tc.For_i_unrolled(0, n_iters, 1, body, max_unroll=8)
            # Drain: ln of the final iteration's products (the only Ln whose
            # result survives into ob for the K=1 correctness path).
            lnfin = tree.tile([P, FD // G], bf16, tag="lnout")
            nc.scalar.activation(
                lnfin[:],
                prev_t4[0][:],
                mybir.ActivationFunctionType.Ln,
                accum_out=ob[:],
            )
            nc.sync.dma_start(out_ap[:], ob[:])
    nc.compile()
    return nc


def _get_nc(n_iters: int = 1):
    if n_iters not in _CACHE:
        _CACHE[n_iters] = _build_bass(n_iters)
    return _CACHE[n_iters]


def _shard_inputs(pred):
    """fp8 e4m3 per-core shards in the [128, 12288] device layout."""
    import ml_dtypes

    pred8 = np.ascontiguousarray(pred).astype(ml_dtypes.float8_e4m3)
    return [
        {"pred": pred8[c * IMGS_PER_CORE : (c + 1) * IMGS_PER_CORE].reshape(P, FD)}
        for c in range(N_CORES)
    ]


def _device_softplus_sum(pred):
    """Run the 8-core SPMD kernel; return the global sum of softplus(pred)."""
    from concourse.bass_utils import run_bass_kernel_spmd

    nc = _get_nc(1)
    in_maps = _shard_inputs(pred)
    res = run_bass_kernel_spmd(nc, in_maps, list(range(N_CORES))).results
    # device accumulated ln(prod (1+e^x)/2) = sum softplus(x) - count*ln2
    dev = sum(res[c]["out"].astype(np.float64).sum() for c in range(N_CORES))
    return float(dev) + B * H * W * LN2


def kernel(pred, target, hann_kernel):
    pred = np.asarray(pred, dtype=np.float32)
    target = np.asarray(target, dtype=np.float32)
    hann = np.asarray(hann_kernel, dtype=np.float32)

    sp_global = _device_softplus_sum(pred)

    hann64 = hann.astype(np.float64)
    nzmask = hann64 != 0.0
    S = hann64.sum()
    n_zero = H * W - int(nzmask.sum())

    # Box top-left: first 1.0 of each image in row-major order (0,0 if none).
    flat_idx = np.argmax(target.reshape(B, -1) == 1.0, axis=1)
    tail = 0.0
    for i in range(B):
        y0, x0 = divmod(int(flat_idx[i]), W)
        # dynamic_update_slice clamps the window to stay in-bounds
        y0 = min(y0, H - KW)
        x0 = min(x0, W - KW)
        pp = pred[i, y0 : y0 + KW, x0 : x0 + KW].astype(np.float64)
        tt = target[i, y0 : y0 + KW, x0 : x0 + KW].astype(np.float64)
        pt_box = pp * tt
        bce_box = np.logaddexp(0.0, pp) - pt_box
        A = (bce_box * hann64).sum()
        Z = bce_box[nzmask].sum()
        tail += A / (2.0 * S) - (Z + pt_box.sum()) / (2.0 * n_zero)

    loss = tail / B + (sp_global / B) / (2.0 * n_zero)
    return np.array(loss, dtype=np.float32)


# revision 12
# speedup vs baseline: 2.0935x; 1.2166x over previous
"""Weighted-BCE (Hanning) loss on 8 Trainium2 NeuronCores.

Math: reference loss per image i with box top-left (y0,x0) (the 33x33 block of
1.0s in target; (0,0) when absent) and hann window h (S = sum(h), nnz = count
of h != 0, n_zero = H*W - nnz):

    weights = h/(2S) on box positions where h != 0, else 1/(2*n_zero)
    bce     = softplus(pred) - pred*target
    loss_i  = sum_box(bce*h)/(2S) + (T_i - Z_i)/(2*n_zero)
      T_i   = sum_all(softplus(pred)) - sum_box(pred)
      Z_i   = sum_box(bce * (h != 0))

Only mean_i(loss_i) is required, and mean_i T_i depends only on the GLOBAL
softplus sum, so the device computes exactly one O(B*H*W) quantity: the sum of
softplus(pred) over every pixel. The box tail (A_i, Z_i, sum_box(pred)) is
O(B*33^2) and computed on host, with the box located by a single argmax over
target (first 1.0 in row-major order is the box's top-left corner; (0,0) when
target is all zeros, matching the reference's argmax-of-zeros behavior).

Device kernel per core (pure data parallel, 6 images per core, 1,572,864
elements as fp8 e4m3 viewed as [128, 12288]):

There is no hardware Softplus activation table, so two full ACT table passes
(Exp then Ln) would cost ~2N scalar-engine cycles. Instead use
    sum_j ln(1+e^{x_j}) = ln(prod_j (1+e^{x_j}))
over groups of G=8: the scalar engine's ONLY instruction is one full-N Exp
pass v = exp(x - ln2) = e^x/2 (the /2 centers group products near 1 so bf16
range is safe). The vector engine does everything else in its fast perf
modes: w = v + 0.5 (tensor_scalar, 4x), a 3-level pairwise product tree
(raw InstTensorTensor mult, the only 2x-capable elementwise op; the bass
scalar_tensor_tensor wrapper runs at 1x), and the final group ln via a
bit trick: for positive bf16 p, bits(p) as int16 gives
ln(p) ~= I*(ln2/128) + ln2*(0.05730496 - 127), exact in the exponent and
mean-centered in the mantissa (classic frac-linearization; measured bias
on the product distribution ~5 units in 1e7). That ln is one int16-in
tensor_scalar affine (4x) with fused f32 accum_out per-partition sum.
Host adds back the N*ln2 rescale. Per iteration: ACT ~N cycles @1.2GHz
(10.5us) vs DVE ~0.78N @0.96GHz (9.6us) vs DMA 1.5MB (~5us) - ACT-bound.

Value ranges (x ~ N(0,1), |x| <= 5.5 after fp8): v, w in [2e-3, 123] fit
f16; L1 pair products <= 1.5e4 fit f16; L2/L3 in bf16 (8-bit exponent);
f16 is used before that for its finer mantissa (the +0.5 and first product
round 4x finer than bf16, which measurably cuts the accumulated bias).
"""

import numpy as np

B, H, W, KW = 48, 512, 512, 33
N_CORES = 8
IMGS_PER_CORE = B // N_CORES  # 6
FLAT = IMGS_PER_CORE * H * W  # 1,572,864 elements per core
P = 128
FD = FLAT // P  # 12288 elements per partition
G = 8  # product-group size
LN2 = 0.6931471805599453
# bitcast-ln affine: ln(p) ~= int16bits(bf16 p) * LN2/128 + LN_C
LN_C = (0.0573049591110366 - 127.0) * LN2

_CACHE = {}


def _build_bass(n_iters: int = 1):
    """Build+compile the per-core bass program. n_iters>1 repeats the body
    (same inputs) for wall-clock device timing; outputs are identical."""
    import concourse.bass as bass
    import concourse.tile as tile
    from concourse import bacc, mybir

    f32 = mybir.dt.float32
    bf16 = mybir.dt.bfloat16
    f16 = mybir.dt.float16
    i16 = mybir.dt.int16
    f8 = mybir.dt.float8e4
    mult = mybir.AluOpType.mult
    add = mybir.AluOpType.add
    nc = bacc.Bacc("TRN2", target_bir_lowering=False, debug=False, num_devices=N_CORES)
    # Register -ln2 as a const AP so it can be used as an activation bias.
    _c = nc.alloc_sbuf_tensor("const-f32-negln2", [128, 1], f32)
    nc.gpsimd.memset(_c.ap(), -LN2)
    nc.const_aps.aps[(f32, -LN2)] = _c.ap()
    nc.all_engine_barrier()
    pred_ap = nc.dram_tensor("pred", [P, FD], f8, kind="ExternalInput").ap()
    out_ap = nc.dram_tensor("out", [P, 1], f32, kind="ExternalOutput").ap()

    def tt_mult(out, in0, in1):
        """Raw InstTensorTensor multiply - the only 2x-mode elementwise
        two-tensor op (bass's scalar_tensor_tensor lowers to
        InstTensorScalarPtr, which has no fast-mode uops)."""
        return nc.vector.add_instruction(
            mybir.InstTensorTensor(
                name=nc.get_next_instruction_name(),
                op=mult,
                ins=[nc.vector.lower_ap(in0), nc.vector.lower_ap(in1)],
                outs=[nc.vector.lower_ap(out)],
            )
        )

    with tile.TileContext(nc) as tc:
        with (
            tc.tile_pool(name="pin", bufs=2) as pin,
            tc.tile_pool(name="vbuf", bufs=2) as vbuf,
            tc.tile_pool(name="tree", bufs=2) as tree,
            tc.tile_pool(name="obuf", bufs=1) as obuf,
        ):
            ob = obuf.tile([P, 1], f32)

            def body(_iv):
                tx = pin.tile([P, FD], f8, tag="pred")
                nc.sync.dma_start(tx[:], pred_ap[:])
                v = vbuf.tile([P, FD], f16, tag="v")
                # ACT's single pass: v = exp(x - ln2) = e^x / 2
                nc.scalar.activation(
                    v[:], tx[:], mybir.ActivationFunctionType.Exp, bias=-LN2
                )
                # w = v + 0.5 = (1+e^x)/2, in place (tensor_scalar 4x)
                nc.vector.tensor_scalar_add(v[:], v[:], 0.5)
                h = FD // 2
                t1 = tree.tile([P, h], f16, tag="t1")
                tt_mult(t1[:], v[:, :h], v[:, h:])
                h //= 2
                t2 = tree.tile([P, h], bf16, tag="t2")
                tt_mult(t2[:], t1[:, :h], t1[:, h:])
                h //= 2
                t3 = tree.tile([P, h], bf16, tag="t3")
                tt_mult(t3[:], t2[:, :h], t2[:, h:])
                # group ln + per-partition sum in one 4x tensor_scalar:
                # ln(p) ~= bits(p)*(LN2/128) + LN_C. With accum_out, op1 is
                # the REDUCE op and scalar2 its init: ob = 0 + sum(I*s1).
                # The +LN_C per group is a known constant, added on host.
                lnv = tree.tile([P, h], f16, tag="lnv")
                nc.vector.tensor_scalar(
                    lnv[:],
                    t3[:].bitcast(i16),
                    LN2 / 128.0,
                    0.0,
                    mult,
                    add,
                    accum_out=ob[:],
                )

            if n_iters == 1:
                body(0)
            else:
                tc.For_i_unrolled(0, n_iters, 1, body, max_unroll=8)
            nc.sync.dma_start(out_ap[:], ob[:])
    nc.compile()
    return nc


def _get_nc(n_iters: int = 1):
    if n_iters not in _CACHE:
        _CACHE[n_iters] = _build_bass(n_iters)
    return _CACHE[n_iters]


def _shard_inputs(pred):
    """fp8 e4m3 per-core shards in the [128, 12288] device layout."""
    import ml_dtypes

    pred8 = np.ascontiguousarray(pred).astype(ml_dtypes.float8_e4m3)
    return [
        {"pred": pred8[c * IMGS_PER_CORE : (c + 1) * IMGS_PER_CORE].reshape(P, FD)}
        for c in range(N_CORES)
    ]


def _device_softplus_sum(pred):
    """Run the 8-core SPMD kernel; return the global sum of softplus(pred)."""
    from concourse.bass_utils import run_bass_kernel_spmd

    nc = _get_nc(1)
    in_maps = _shard_inputs(pred)
    res = run_bass_kernel_spmd(nc, in_maps, list(range(N_CORES))).results
    # device accumulated sum of bits(prod)*(ln2/128); add the per-group LN_C
    # affine constant and undo the (1+e^x)/2 rescale: + N*ln2
    dev = sum(res[c]["out"].astype(np.float64).sum() for c in range(N_CORES))
    n = B * H * W
    return float(dev) + (n // G) * LN_C + n * LN2


def kernel(pred, target, hann_kernel):
    pred = np.asarray(pred, dtype=np.float32)
    target = np.asarray(target, dtype=np.float32)
    hann = np.asarray(hann_kernel, dtype=np.float32)

    sp_global = _device_softplus_sum(pred)

    hann64 = hann.astype(np.float64)
    nzmask = hann64 != 0.0
    S = hann64.sum()
    n_zero = H * W - int(nzmask.sum())

    # Box top-left: first 1.0 of each image in row-major order (0,0 if none).
    flat_idx = np.argmax(target.reshape(B, -1) == 1.0, axis=1)
    tail = 0.0
    for i in range(B):
        y0, x0 = divmod(int(flat_idx[i]), W)
        # dynamic_update_slice clamps the window to stay in-bounds
        y0 = min(y0, H - KW)
        x0 = min(x0, W - KW)
        pp = pred[i, y0 : y0 + KW, x0 : x0 + KW].astype(np.float64)
        tt = target[i, y0 : y0 + KW, x0 : x0 + KW].astype(np.float64)
        pt_box = pp * tt
        bce_box = np.logaddexp(0.0, pp) - pt_box
        A = (bce_box * hann64).sum()
        Z = bce_box[nzmask].sum()
        tail += A / (2.0 * S) - (Z + pt_box.sum()) / (2.0 * n_zero)

    loss = tail / B + (sp_global / B) / (2.0 * n_zero)
    return np.array(loss, dtype=np.float32)


# revision 16
# speedup vs baseline: 2.2170x; 1.0590x over previous
"""Weighted-BCE (Hanning) loss on 8 Trainium2 NeuronCores.

Math: reference loss per image i with box top-left (y0,x0) (the 33x33 block of
1.0s in target; (0,0) when absent) and hann window h (S = sum(h), nnz = count
of h != 0, n_zero = H*W - nnz):

    weights = h/(2S) on box positions where h != 0, else 1/(2*n_zero)
    bce     = softplus(pred) - pred*target
    loss_i  = sum_box(bce*h)/(2S) + (T_i - Z_i)/(2*n_zero)
      T_i   = sum_all(softplus(pred)) - sum_box(pred)
      Z_i   = sum_box(bce * (h != 0))

Only mean_i(loss_i) is required, and mean_i T_i depends only on the GLOBAL
softplus sum, so the device computes exactly one O(B*H*W) quantity: the sum of
softplus(pred) over every pixel. The box tail (A_i, Z_i, sum_box(pred)) is
O(B*33^2) and computed on host, with the box located by a single argmax over
target (first 1.0 in row-major order is the box's top-left corner; (0,0) when
target is all zeros, matching the reference's argmax-of-zeros behavior).

Device kernel per core (pure data parallel, 6 images per core, 1,572,864
elements as fp8 e4m3 viewed as [128, 12288]). Uses the identity

    softplus(x) = x - ln(sigmoid(x)),  sum ln sig over a group = ln prod sig

so each engine does what it is uniquely fast at, once per element:

- ScalarE (the bottleneck at 1 elem/cycle/lane, 1.2GHz): ONE table pass,
  sig = sigmoid(x), fp8 in -> f16 out. No second table set is ever needed,
  so no per-iteration ACT_TABLE_LOADs.
- VectorE: 3-level pairwise product tree on sig (raw InstTensorTensor mult,
  the only 2x-mode elementwise op; bass's scalar_tensor_tensor wrapper runs
  at 1x), L1 in f16 (pair underflow probability ~1e-12 for N(0,1) data),
  L2/L3 in bf16 (8-bit exponent; group-of-8 products reach e^-18). Then the
  group ln via a bit trick: for positive bf16 p, int16 bits give
  ln(p) ~= I*(ln2/128) + ln2*(0.05730496-127) - exact in the exponent,
  mean-centered in the mantissa. One int16-in tensor_scalar affine (4x) with
  f32 accum_out computes all group lns AND the per-partition sum; the
  constant term is folded on host (known group count).
- TensorE (otherwise idle): sum(x) via 96 ones-vector matmuls [128,128]fp8
  accumulated into one PSUM [128,1] f32; a post-loop copy lands it in SBUF.

Per iteration: ACT ~12.3k cycles @1.2GHz (10.5us) vs DVE ~6k cycles @0.96
(6.3us) vs TensorE ~12.4k @2.4 (5.2us) vs DMA 1.5MB @~360GB/s (4.3us).
ACT-bound; everything else hides underneath with double-buffered pools.
"""

import numpy as np

B, H, W, KW = 48, 512, 512, 33
N_CORES = 8
IMGS_PER_CORE = B // N_CORES  # 6
FLAT = IMGS_PER_CORE * H * W  # 1,572,864 elements per core
P = 128
FD = FLAT // P  # 12288 elements per partition
G = 8  # product-group size
MM = FD // P  # 96 matmul chunks of 128 columns
LN2 = 0.6931471805599453
# bitcast-ln affine: ln(p) ~= int16bits(bf16 p) * LN2/128 + LN_C
LN_C = (0.0573049591110366 - 127.0) * LN2

_CACHE = {}


def _build_bass(n_iters: int = 1):
    """Build+compile the per-core bass program. n_iters>1 repeats the body
    (same inputs) for wall-clock device timing; outputs are identical."""
    import os

    import concourse.bass as bass
    import concourse.tile as tile
    from concourse import bacc, mybir

    f32 = mybir.dt.float32
    bf16 = mybir.dt.bfloat16
    f16 = mybir.dt.float16
    i16 = mybir.dt.int16
    f8 = mybir.dt.float8e4
    mult = mybir.AluOpType.mult
    add = mybir.AluOpType.add
    nc = bacc.Bacc("TRN2", target_bir_lowering=False, debug=False, num_devices=N_CORES)
    pred_ap = nc.dram_tensor("pred", [P, FD], f8, kind="ExternalInput").ap()
    outln_ap = nc.dram_tensor("outln", [P, 1], f32, kind="ExternalOutput").ap()
    outx_ap = nc.dram_tensor("outx", [P, 1], f32, kind="ExternalOutput").ap()

    def tt_mult(out, in0, in1):
        """Raw InstTensorTensor multiply - the only 2x-mode elementwise
        two-tensor op (bass's scalar_tensor_tensor lowers to
        InstTensorScalarPtr, which has no fast-mode uops)."""
        return nc.vector.add_instruction(
            mybir.InstTensorTensor(
                name=nc.get_next_instruction_name(),
                op=mult,
                ins=[nc.vector.lower_ap(in0), nc.vector.lower_ap(in1)],
                outs=[nc.vector.lower_ap(out)],
            )
        )

    n_stages = int(os.environ.get("KERNEL_STAGES", "6"))

    with tile.TileContext(nc) as tc:
        with (
            tc.tile_pool(name="pin", bufs=2) as pin,
            tc.tile_pool(name="vbuf", bufs=2) as vbuf,
            tc.tile_pool(name="tree", bufs=2) as tree,
            tc.tile_pool(name="psum", bufs=1, space="PSUM") as psum,
            tc.tile_pool(name="obuf", bufs=1) as obuf,
        ):
            obln = obuf.tile([P, 1], f32)
            nc.vector.memset(obln[:], 0.0)
            ones8 = obuf.tile([P, 1], f8)
            nc.vector.memset(ones8[:], 1.0)
            px = psum.tile([P, 1], f32)

            def body(_iv):
                tx = pin.tile([P, FD], f8, tag="pred")
                nc.sync.dma_start(tx[:], pred_ap[:])
                if n_stages < 2:
                    return
                sg = vbuf.tile([P, FD], f16, tag="sg")
                # ACT's single pass: sg = sigmoid(x)
                nc.scalar.activation(
                    sg[:], tx[:], mybir.ActivationFunctionType.Sigmoid
                )
                if n_stages >= 4:
                    # TensorE: px += tx_chunk.T @ ones for 96 column chunks;
                    # px[m] accumulates sum of all x in columns == m (mod 128)
                    for k in range(MM):
                        nc.tensor.matmul(
                            px[:],
                            tx[:, k * P : (k + 1) * P],
                            ones8[:],
                            start=(k == 0),
                            stop=(k == MM - 1),
                        )
                if n_stages < 3:
                    return
                h = FD // 2
                t1 = tree.tile([P, h], f16, tag="t1")
                tt_mult(t1[:], sg[:, :h], sg[:, h:])
                h //= 2
                t2 = tree.tile([P, h], bf16, tag="t2")
                tt_mult(t2[:], t1[:, :h], t1[:, h:])
                h //= 2
                t3 = tree.tile([P, h], bf16, tag="t3")
                tt_mult(t3[:], t2[:, :h], t2[:, h:])
                if n_stages < 5:
                    return
                # group ln + per-partition sum in one 4x tensor_scalar:
                # ln(p) ~= bits(p)*(LN2/128) + LN_C. With accum_out, op1 is
                # the REDUCE op and scalar2 its init: obln = 0 + sum(I*s1).
                # The +LN_C per group is a known constant, added on host.
                lnv = tree.tile([P, h], f16, tag="lnv")
                nc.vector.tensor_scalar(
                    lnv[:],
                    t3[:].bitcast(i16),
                    LN2 / 128.0,
                    0.0,
                    mult,
                    add,
                    accum_out=obln[:],
                )

            if n_iters == 1:
                body(0)
            else:
                tc.For_i_unrolled(0, n_iters, 1, body, max_unroll=8)
            obx = obuf.tile([P, 1], f32)
            if n_stages >= 4:
                nc.vector.tensor_copy(obx[:], px[:])
            else:
                nc.vector.memset(obx[:], 0.0)
            nc.sync.dma_start(outln_ap[:], obln[:])
            nc.sync.dma_start(outx_ap[:], obx[:])
    nc.compile()
    return nc


def _get_nc(n_iters: int = 1):
    if n_iters not in _CACHE:
        _CACHE[n_iters] = _build_bass(n_iters)
    return _CACHE[n_iters]


def _shard_inputs(pred):
    """fp8 e4m3 per-core shards in the [128, 12288] device layout."""
    import ml_dtypes

    pred8 = np.ascontiguousarray(pred).astype(ml_dtypes.float8_e4m3)
    return [
        {"pred": pred8[c * IMGS_PER_CORE : (c + 1) * IMGS_PER_CORE].reshape(P, FD)}
        for c in range(N_CORES)
    ]


def _device_softplus_sum(pred):
    """Run the 8-core SPMD kernel; return the global sum of softplus(pred)."""
    from concourse.bass_utils import run_bass_kernel_spmd

    nc = _get_nc(1)
    in_maps = _shard_inputs(pred)
    res = run_bass_kernel_spmd(nc, in_maps, list(range(N_CORES))).results
    # sum softplus(x) = sum(x) - sum(ln sigmoid(x)); the device ships
    # sum(x) (outx) and sum over groups of bits(prod sig)*(ln2/128) (outln),
    # to which the per-group affine constant LN_C is added here.
    lnsum = sum(res[c]["outln"].astype(np.float64).sum() for c in range(N_CORES))
    xsum = sum(res[c]["outx"].astype(np.float64).sum() for c in range(N_CORES))
    n = B * H * W
    return xsum - (lnsum + (n // G) * LN_C)


def kernel(pred, target, hann_kernel):
    pred = np.asarray(pred, dtype=np.float32)
    target = np.asarray(target, dtype=np.float32)
    hann = np.asarray(hann_kernel, dtype=np.float32)

    sp_global = _device_softplus_sum(pred)

    hann64 = hann.astype(np.float64)
    nzmask = hann64 != 0.0
    S = hann64.sum()
    n_zero = H * W - int(nzmask.sum())

    # Box top-left: first 1.0 of each image in row-major order (0,0 if none).
    flat_idx = np.argmax(target.reshape(B, -1) == 1.0, axis=1)
    tail = 0.0
    for i in range(B):
        y0, x0 = divmod(int(flat_idx[i]), W)
        # dynamic_update_slice clamps the window to stay in-bounds
        y0 = min(y0, H - KW)
        x0 = min(x0, W - KW)
        pp = pred[i, y0 : y0 + KW, x0 : x0 + KW].astype(np.float64)
        tt = target[i, y0 : y0 + KW, x0 : x0 + KW].astype(np.float64)
        pt_box = pp * tt
        bce_box = np.logaddexp(0.0, pp) - pt_box
        A = (bce_box * hann64).sum()
        Z = bce_box[nzmask].sum()
        tail += A / (2.0 * S) - (Z + pt_box.sum()) / (2.0 * n_zero)

    loss = tail / B + (sp_global / B) / (2.0 * n_zero)
    return np.array(loss, dtype=np.float32)


# revision 18
# speedup vs baseline: 2.7800x; 1.2540x over previous
"""Weighted-BCE (Hanning) loss on 8 Trainium2 NeuronCores.

Math: reference loss per image i with box top-left (y0,x0) (the 33x33 block of
1.0s in target; (0,0) when absent) and hann window h (S = sum(h), nnz = count
of h != 0, n_zero = H*W - nnz):

    weights = h/(2S) on box positions where h != 0, else 1/(2*n_zero)
    bce     = softplus(pred) - pred*target
    loss_i  = sum_box(bce*h)/(2S) + (T_i - Z_i)/(2*n_zero)
      T_i   = sum_all(softplus(pred)) - sum_box(pred)
      Z_i   = sum_box(bce * (h != 0))

Only mean_i(loss_i) is required, and mean_i T_i depends only on the GLOBAL
softplus sum, so the device computes exactly one O(B*H*W) quantity: the sum of
softplus(pred) over every pixel. The box tail (A_i, Z_i, sum_box(pred)) is
O(B*33^2) and computed on host, with the box located by a single argmax over
target (first 1.0 in row-major order is the box's top-left corner; (0,0) when
target is all zeros, matching the reference's argmax-of-zeros behavior).

Device kernel per core (pure data parallel, 6 images per core, 1,572,864
elements viewed as [128, 12288]). The scalar engine is the structural
bottleneck (1 elem/cycle/lane), so its per-element work is pushed to ONE
sigmoid table pass over only the first MAIN columns; the last OFF columns
never touch ACT at all - they go through a vector-engine fast-exp bit trick.

MAIN share -- softplus(x) = x - ln(sigmoid(x)):
- ScalarE: sig = sigmoid(x), fp8 in -> f16 out, one pass, one table set.
- TensorE (otherwise idle): sum(x) over the MAIN columns via ones-vector
  matmuls ([128,128] fp8 stationary x ones moving) accumulated in PSUM.
- VectorE: 3-level pairwise product tree on sig (raw InstTensorTensor mult,
  the only 2x-mode elementwise op; bass's scalar_tensor_tensor wrapper runs
  at 1x). L1 f16, L2/L3 bf16. Then ln(prod of 8) for every group via the
  bitcast trick: for positive bf16 p, int16 bits give
  ln(p) ~= I*(ln2/128) + ln2*(0.05730496-127) - exact in the exponent,
  mean-centered in the mantissa - computed+row-summed by one int16-in 4x
  tensor_scalar with f32 accum_out.

OFF share -- softplus(x) = ln((1+e^x)/2) + ln2, with e^x/2 from bits:
- f16 input (ships at 2B/elem); I = int16(x*1024*log2e + 1024*14 + magic)
  bitcast as f16 IS 2^(x*log2e - 1) ~= e^x/2 (magic = -58.68 centers the
  mantissa-linearization sawtooth; calibrated bias ~4e-5 relative).
- +0.5 (4x tensor_scalar, in place on the f16 view), its own small product
  tree, same bitcast-ln into a second accumulator column.

Per iteration at OFF = 20 chunks (2560 cols): ACT 9728 elem (8.4us), DVE
~4.7+2.7us, TensorE 76 matmuls (4.3us), DMA 1.22MB fp8 + 0.65MB f16
(~5.3us). The For_i hardware loop is unrolled x32 - each loop back-edge
costs a pipeline drain (~17us per edge, measured via unroll ablation).
"""

import os

import numpy as np

B, H, W, KW = 48, 512, 512, 33
N_CORES = 8
IMGS_PER_CORE = B // N_CORES  # 6
FLAT = IMGS_PER_CORE * H * W  # 1,572,864 elements per core
P = 128
FD = FLAT // P  # 12288 elements per partition
G = 8  # product-group size
LN2 = 0.6931471805599453
LOG2E = 1.4426950408889634
# bitcast-ln affine: ln(p) ~= int16bits(bf16 p) * LN2/128 + LN_C
LN_C = (0.0573049591110366 - 127.0) * LN2
FAST_MAGIC = -58.68  # = -0.0573*1024, centers the fast-exp sawtooth

OFF_CH = int(os.environ.get("KERNEL_OFF_CH", "20"))  # 128-col chunks on DVE
OC = OFF_CH * P
MAIN = FD - OC

_CACHE = {}


def _build_bass(n_iters: int = 1):
    """Build+compile the per-core bass program. n_iters>1 repeats the body
    (same inputs) for wall-clock device timing; outputs are identical."""
    import concourse.bass as bass
    import concourse.tile as tile
    from concourse import bacc, mybir

    f32 = mybir.dt.float32
    bf16 = mybir.dt.bfloat16
    f16 = mybir.dt.float16
    i16 = mybir.dt.int16
    f8 = mybir.dt.float8e4
    mult = mybir.AluOpType.mult
    add = mybir.AluOpType.add
    nc = bacc.Bacc("TRN2", target_bir_lowering=False, debug=False, num_devices=N_CORES)
    pred_ap = nc.dram_tensor("pred", [P, MAIN], f8, kind="ExternalInput").ap()
    if OC:
        predh_ap = nc.dram_tensor("predh", [P, OC], f16, kind="ExternalInput").ap()
    outln_ap = nc.dram_tensor("outln", [P, 2], f32, kind="ExternalOutput").ap()
    outx_ap = nc.dram_tensor("outx", [P, 1], f32, kind="ExternalOutput").ap()

    def tt_mult(out, in0, in1):
        """Raw InstTensorTensor multiply - the only 2x-mode elementwise
        two-tensor op (bass's scalar_tensor_tensor lowers to
        InstTensorScalarPtr, which has no fast-mode uops)."""
        return nc.vector.add_instruction(
            mybir.InstTensorTensor(
                name=nc.get_next_instruction_name(),
                op=mult,
                ins=[nc.vector.lower_ap(in0), nc.vector.lower_ap(in1)],
                outs=[nc.vector.lower_ap(out)],
            )
        )

    with tile.TileContext(nc) as tc:
        with (
            tc.tile_pool(name="pin", bufs=2) as pin,
            tc.tile_pool(name="vbuf", bufs=2) as vbuf,
            tc.tile_pool(name="tree", bufs=2) as tree,
            tc.tile_pool(name="psum", bufs=1, space="PSUM") as psum,
            tc.tile_pool(name="obuf", bufs=1) as obuf,
        ):
            obln = obuf.tile([P, 2], f32)
            nc.vector.memset(obln[:], 0.0)
            ones8 = obuf.tile([P, 1], f8)
            nc.vector.memset(ones8[:], 1.0)
            px = psum.tile([P, 1], f32)

            def ln_accum(t3ap, ncols, acc, tag):
                """ln of positive bf16 group products + per-partition row sum
                in one 4x tensor_scalar: out = bits*(LN2/128); with accum_out,
                op1 is the REDUCE op and scalar2 its init: acc = 0 + sum(out).
                The +LN_C affine constant is folded on host."""
                lnv = tree.tile([P, ncols], f16, tag=tag)
                nc.vector.tensor_scalar(
                    lnv[:], t3ap.bitcast(i16), LN2 / 128.0, 0.0, mult, add,
                    accum_out=acc,
                )

            def body(_iv):
                tx = pin.tile([P, MAIN], f8, tag="pred")
                nc.sync.dma_start(tx[:], pred_ap[:])
                if OC:
                    txh = pin.tile([P, OC], f16, tag="predh")
                    nc.sync.dma_start(txh[:], predh_ap[:])
                    # fast-exp: I = int16(x*1024*log2e + (1024*14 + magic));
                    # I bitcast as f16 == 2^(x*log2e - 1) ~= e^x/2
                    fe = vbuf.tile([P, OC], i16, tag="fe")
                    nc.vector.tensor_scalar(
                        fe[:],
                        txh[:],
                        1024.0 * LOG2E,
                        1024.0 * 14.0 + FAST_MAGIC,
                        mult,
                        add,
                    )
                    few = fe[:].bitcast(f16)
                    nc.vector.tensor_scalar_add(few, few, 0.5)  # (1+e^x)/2
                    ho = OC // 2
                    o1 = tree.tile([P, ho], f16, tag="o1")
                    tt_mult(o1[:], few[:, :ho], few[:, ho:])
                    ho //= 2
                    o2 = tree.tile([P, ho], bf16, tag="o2")
                    tt_mult(o2[:], o1[:, :ho], o1[:, ho:])
                    ho //= 2
                    o3 = tree.tile([P, ho], bf16, tag="o3")
                    tt_mult(o3[:], o2[:, :ho], o2[:, ho:])
                    ln_accum(o3[:], ho, obln[:, 1:2], "lnoff")
                sg = vbuf.tile([P, MAIN], f16, tag="sg")
                # ACT's single pass: sg = sigmoid(x) over the MAIN columns
                nc.scalar.activation(
                    sg[:], tx[:], mybir.ActivationFunctionType.Sigmoid
                )
                # TensorE: px += tx_chunk.T @ ones per 128-col chunk;
                # px[m] accumulates sum of x over columns == m (mod 128)
                nmm = MAIN // P
                for k in range(nmm):
                    nc.tensor.matmul(
                        px[:],
                        tx[:, k * P : (k + 1) * P],
                        ones8[:],
                        start=(k == 0),
                        stop=(k == nmm - 1),
                    )
                h = MAIN // 2
                t1 = tree.tile([P, h], f16, tag="t1")
                tt_mult(t1[:], sg[:, :h], sg[:, h:])
                h //= 2
                t2 = tree.tile([P, h], bf16, tag="t2")
                tt_mult(t2[:], t1[:, :h], t1[:, h:])
                h //= 2
                t3 = tree.tile([P, h], bf16, tag="t3")
                tt_mult(t3[:], t2[:, :h], t2[:, h:])
                ln_accum(t3[:], h, obln[:, 0:1], "lnmain")

            if n_iters == 1:
                body(0)
            else:
                tc.For_i_unrolled(
                    0, n_iters, 1, body,
                    max_unroll=int(os.environ.get("KERNEL_UNROLL", "32")),
                )
            obx = obuf.tile([P, 1], f32)
            nc.vector.tensor_copy(obx[:], px[:])
            nc.sync.dma_start(outln_ap[:], obln[:])
            nc.sync.dma_start(outx_ap[:], obx[:])
    nc.compile()
    return nc


def _get_nc(n_iters: int = 1):
    if n_iters not in _CACHE:
        _CACHE[n_iters] = _build_bass(n_iters)
    return _CACHE[n_iters]


def _shard_inputs(pred):
    """Per-core shards: first MAIN cols as fp8 e4m3, last OC cols as f16."""
    import ml_dtypes

    flat = np.ascontiguousarray(pred).reshape(N_CORES, P, FD)
    maps = []
    for c in range(N_CORES):
        m = {
            "pred": np.ascontiguousarray(flat[c, :, :MAIN]).astype(
                ml_dtypes.float8_e4m3
            )
        }
        if OC:
            m["predh"] = np.ascontiguousarray(flat[c, :, MAIN:]).astype(np.float16)
        maps.append(m)
    return maps


def _device_softplus_sum(pred):
    """Run the 8-core SPMD kernel; return the global sum of softplus(pred)."""
    from concourse.bass_utils import run_bass_kernel_spmd

    nc = _get_nc(1)
    in_maps = _shard_inputs(pred)
    res = run_bass_kernel_spmd(nc, in_maps, list(range(N_CORES))).results
    ln_main = sum(
        res[c]["outln"][:, 0].astype(np.float64).sum() for c in range(N_CORES)
    )
    ln_off = sum(
        res[c]["outln"][:, 1].astype(np.float64).sum() for c in range(N_CORES)
    )
    xsum = sum(res[c]["outx"].astype(np.float64).sum() for c in range(N_CORES))
    n_main = N_CORES * P * MAIN
    n_off = N_CORES * P * OC
    # main: sum softplus = sum(x) - sum(ln sigmoid)
    # off:  sum softplus = sum(ln (1+e^x)/2) + n*ln2
    sp = xsum - (ln_main + (n_main // G) * LN_C)
    if OC:
        sp += (ln_off + (n_off // G) * LN_C) + n_off * LN2
    return sp


def kernel(pred, target, hann_kernel):
    pred = np.asarray(pred, dtype=np.float32)
    target = np.asarray(target, dtype=np.float32)
    hann = np.asarray(hann_kernel, dtype=np.float32)

    sp_global = _device_softplus_sum(pred)

    hann64 = hann.astype(np.float64)
    nzmask = hann64 != 0.0
    S = hann64.sum()
    n_zero = H * W - int(nzmask.sum())

    # Box top-left: first 1.0 of each image in row-major order (0,0 if none).
    flat_idx = np.argmax(target.reshape(B, -1) == 1.0, axis=1)
    tail = 0.0
    for i in range(B):
        y0, x0 = divmod(int(flat_idx[i]), W)
        # dynamic_update_slice clamps the window to stay in-bounds
        y0 = min(y0, H - KW)
        x0 = min(x0, W - KW)
        pp = pred[i, y0 : y0 + KW, x0 : x0 + KW].astype(np.float64)
        tt = target[i, y0 : y0 + KW, x0 : x0 + KW].astype(np.float64)
        pt_box = pp * tt
        bce_box = np.logaddexp(0.0, pp) - pt_box
        A = (bce_box * hann64).sum()
        Z = bce_box[nzmask].sum()
        tail += A / (2.0 * S) - (Z + pt_box.sum()) / (2.0 * n_zero)

    loss = tail / B + (sp_global / B) / (2.0 * n_zero)
    return np.array(loss, dtype=np.float32)
